# revision 29
# baseline (speedup 1.0000x reference)
"""Trainium2 Bass kernel for DeepSelfAttention (N=8192, D=1024) on 8 NeuronCores.

Strategy (row-parallel attention, fp8 DoubleRow matmuls):
  - Shard the N=8192 rows of x across 8 cores (1024 rows each); replicate
    weights.  x and the Q/K/V weights are pre-transposed and cast to fp8-e4m3
    on the HOST; the MLP weights stay fp16.  Every matmul operand is DMA'd
    directly into its contraction-major SBUF layout.
  - Bias algebra (host-folded):
      * bk drops out of softmax entirely (it shifts every score of a row by a
        q-dependent constant).
      * bv is folded into the first MLP bias: b1' = b1 + W1 @ bv (softmax rows
        sum to 1).
    Only bq survives on device (added to Q after the projection).
  - All projection / attention matmuls and MLP layer 1 run in fp8 with
    perf_mode=DoubleRow: operands are 3D APs [128, 2, free] contracting 256
    rows per pass, ~1.8x the fp16 matmul rate.  Host-side numpy simulation of
    this exact quantization scheme (fp8 x/Wq/Wk/Wv/W1/Q/K/V/exp, fp16 MLP
    L2/L3) gives max rel err 6.7e-3 vs the fp64 reference (gate is 2e-2);
    measured on hardware: 7.3e-3.  Full-fp8 MLP would give 2.8e-2 -> L2/L3
    stay fp16.
  - Each core computes K^T and V for its row shard and ships them in four
    fp8 chunks of [1, 1, 2, 4] key-tiles ([p][K-line|V-line]-interleaved so
    the consumer fetches a unit as ONE dma with 2KB-contiguous partition
    lines); each chunk is AllGathered as soon as it is ready.
  - The first collective carries a fixed ~50-85us rendezvous latency (core
    dispatch skew), so attention group 0 processes the core's OWN 8 key
    units straight from SBUF (kts/vs stay resident) inside that window.
    The 56 remote units are then fetched from the gathered buffers,
    skipping the own block via a partition-id-dependent dynamic DMA offset
    kb = (pid+1+j) & 7; remote group sizes (6,8,8,8,8,8,10) are matched to
    chunk arrival times so no group waits on a collective.
  - Attention groups pair units for DoubleRow: scores^T tiles [k=128,
    q=512] accumulate over feature-tile PAIRS in PSUM (4 DR matmuls), exp
    on ScalarE (scale=1/32 fused; no max-subtraction needed, scores are
    small) into paired fp8 tiles [128, 2, 512], softmax denominator via a
    ones-vector DR matmul, A@V accumulated across the group's unit-pairs
    (DR chains) and flushed to an SBUF fp32 accumulator.
  - Normalize via PE broadcast of 1/rowsum (emitted inside the last
    attention group, straight to fp8), then the MLP: fp8-DR layer 1, fp16
    layers 2/3 + final projection, the two 512-query column halves
    interleaved chain-by-chain to hide layer boundaries.
"""

import numpy as np

import concourse.mybir as mybir
import concourse.tile as tile
from concourse import bacc
from concourse.bass import ds
from concourse import bass_utils

P = 128
D = 1024
N = 8192
NCORES = 8
NS = N // NCORES          # 1024 rows per core
DT = D // P               # 8 feature tiles
DT2 = DT // 2             # 4 feature-tile PAIRS (DoubleRow)
KTB = NS // P             # 8 k tiles per block
CHUNK_KTS = [[0], [1], [2, 3], [4, 5, 6, 7]]  # kt split per AllGather chunk
USZ = P * D               # elements per K or V unit (128 keys x 1024 feat)
F16 = mybir.dt.float16
F32 = mybir.dt.float32
F8 = mybir.dt.float8e4
AF = mybir.ActivationFunctionType
ALU = mybir.AluOpType
DR = mybir.MatmulPerfMode.DoubleRow

SCALE = 1.0 / np.sqrt(np.float32(D)).astype(np.float32)  # 0.03125

_CACHE = {}


def _build():
    nc = bacc.Bacc("TRN2", target_bir_lowering=False, debug=False,
                   num_devices=NCORES)
    # All inputs arrive pre-arranged by the host in the exact SBUF layout
    # ([partition, feature-tile, cols]) so every preamble DMA moves fully
    # contiguous 4-8KB partition lines (~2.3x the strided-DMA rate).
    xsT0 = nc.dram_tensor("xsT0", [P, DT, 512], F8, kind="ExternalInput").ap()
    xsT1 = nc.dram_tensor("xsT1", [P, DT, 512], F8, kind="ExternalInput").ap()
    W = {}
    for w in ("wqT", "wkT", "wvT", "w1T"):
        W[w] = nc.dram_tensor(w, [P, DT, D], F8, kind="ExternalInput").ap()
    for w in ("w2T", "w3T"):
        W[w] = nc.dram_tensor(w, [P, DT, D], F16, kind="ExternalInput").ap()
    B = {}
    for b in ("bq", "b1", "b2", "b3"):
        B[b] = nc.dram_tensor(b, [P, DT], F32, kind="ExternalInput").ap()
    fw = nc.dram_tensor("fw", [P, DT], F16, kind="ExternalInput").ap()
    out = nc.dram_tensor("out", [1, NS], F32, kind="ExternalOutput").ap()

    with tile.TileContext(nc) as tc:
        with (
            tc.tile_pool(name="persist", bufs=1) as pers,
            tc.tile_pool(name="dram", bufs=1, space="DRAM") as dram,
        ):
            # ---- persistent SBUF tiles ----
            qt = pers.tile([P, DT, NS], F8, tag="qt")           # Q^T (fp8)
            wT = {"w1T": pers.tile([P, DT, D], F8, tag="w1T", name="w1T")}
            for w in ("w2T", "w3T"):
                wT[w] = pers.tile([P, DT, D], F16, tag=f"{w}", name=f"{w}")
            bsb = {b: pers.tile([P, DT], F32, tag=f"{b}sb", name=f"{b}sb")
                   for b in B}
            fwh = pers.tile([P, DT], F16, tag="fwh")
            ones_p8 = pers.tile([P, 2, 16], F8, tag="ones8")    # DR rowsum
            ones_row = pers.tile([1, P], F16, tag="ones_row")
            rs = pers.tile([1, NS], F32, tag="rs")              # softmax denom
            rs_h = pers.tile([1, NS], F16, tag="rs_h")

            # ---- DRAM scratch: per-chunk flat [kt][K-unit | V-unit] buffers
            csz = [2 * USZ * len(k) for k in CHUNK_KTS]
            kv_d = [dram.tile([csz[c]], F8, name=f"kv_d{c}")
                    for c in range(len(CHUNK_KTS))]
            kvag = [dram.tile([NCORES * csz[c]], F8, name=f"kvag{c}",
                              addr_space="Shared")
                    for c in range(len(CHUNK_KTS))]

            # own-shard K^T / V live through attention: group 0 computes
            # attention on the local block straight from SBUF while the
            # first AllGather is still in flight.  (Allocated before the
            # early pool: pool releases must be LIFO.)
            kvloc = tc.alloc_tile_pool(name="kvloc", bufs=1)
            kts = kvloc.tile([P, DT, NS], F8, tag="kts")        # K^T shard
            vs = kvloc.tile([P, KTB, D], F8, tag="vs")          # V shard
            # ---- early pool: dies after projections ----
            early = tc.alloc_tile_pool(name="early", bufs=1)
            xsb = early.tile([P, DT, NS], F8, tag="xsb")
            wesb = {w: early.tile([P, DT, D], F8, tag=f"{w}", name=f"{w}")
                    for w in ("wqT", "wkT", "wvT")}

            # x (first half first) and the K/V weights lead the DMA queue so
            # the first projection matmul can start as early as possible.
            nc.sync.dma_start(xsb[:, :, 0:512], xsT0[:])
            nc.sync.dma_start(wesb["wkT"][:], W["wkT"][:])
            nc.sync.dma_start(wesb["wvT"][:], W["wvT"][:])
            nc.sync.dma_start(xsb[:, :, 512:1024], xsT1[:])
            nc.sync.dma_start(wesb["wqT"][:], W["wqT"][:])
            for b in B:
                nc.sync.dma_start(bsb[b][:], B[b][:])
            nc.sync.dma_start(fwh[:], fw[:])
            nc.gpsimd.memset(ones_p8[:], 1.0)
            nc.gpsimd.memset(ones_row[:], 1.0)

            def kproj(ppj, h):
                for dt in range(DT):
                    ps = ppj.tile([P, 512], F32, tag="ppj")
                    for e2 in range(DT2):
                        nc.tensor.matmul(
                            ps[:],
                            wesb["wkT"][:, 2 * e2:2 * e2 + 2,
                                        dt * P:(dt + 1) * P],
                            xsb[:, 2 * e2:2 * e2 + 2, h * 512:(h + 1) * 512],
                            start=(e2 == 0), stop=(e2 == DT2 - 1),
                            perf_mode=DR)
                    nc.scalar.activation(
                        kts[:, dt, h * 512:(h + 1) * 512], ps[:], AF.Copy)

            def vproj(ppj, kt):
                for dh in range(2):
                    ps = ppj.tile([P, 512], F32, tag="ppj")
                    for e2 in range(DT2):
                        nc.tensor.matmul(
                            ps[:],
                            xsb[:, 2 * e2:2 * e2 + 2, kt * P:(kt + 1) * P],
                            wesb["wvT"][:, 2 * e2:2 * e2 + 2,
                                        dh * 512:(dh + 1) * 512],
                            start=(e2 == 0), stop=(e2 == DT2 - 1),
                            perf_mode=DR)
                    nc.scalar.activation(
                        vs[:, kt, dh * 512:(dh + 1) * 512], ps[:], AF.Copy)

            def qproj(ppj):
                for dt in range(DT):
                    for h in range(2):
                        ps = ppj.tile([P, 512], F32, tag="ppj")
                        for e2 in range(DT2):
                            nc.tensor.matmul(
                                ps[:],
                                wesb["wqT"][:, 2 * e2:2 * e2 + 2,
                                            dt * P:(dt + 1) * P],
                                xsb[:, 2 * e2:2 * e2 + 2,
                                    h * 512:(h + 1) * 512],
                                start=(e2 == 0), stop=(e2 == DT2 - 1),
                                perf_mode=DR)
                        nc.vector.tensor_tensor(
                            qt[:, dt, h * 512:(h + 1) * 512], ps[:],
                            bsb["bq"][:, dt:dt + 1].to_broadcast([P, 512]),
                            ALU.add)

            def ship(c):
                # Interleaved K|V per partition: each shipped unit is laid
                # out [p][K-line 1024B | V-line 1024B] so the consumer side
                # fetches one unit as a single DMA with 2KB-contiguous
                # partition lines (twice the effective DMA rate of 1KB).
                for u, g in enumerate(CHUNK_KTS[c]):
                    uview = kv_d[c][2 * USZ * u:2 * USZ * (u + 1)].rearrange(
                        "(p x) -> p x", x=2048)
                    nc.sync.dma_start(uview[:, 0:1024],
                                      kts[:, :, g * P:(g + 1) * P])
                    nc.sync.dma_start(uview[:, 1024:2048], vs[:, g, :])
                nc.gpsimd.collective_compute(
                    "AllGather", ALU.bypass,
                    replica_groups=[list(range(NCORES))],
                    ins=[kv_d[c].opt()], outs=[kvag[c].opt()])

            with tc.tile_pool(name="ppj", bufs=4, space="PSUM") as ppj:
                # K^T = Wk @ xs^T (no bias: bk cancels in softmax);
                # V = xs @ Wv.T (bias folded into b1'). K first (its weight
                # slices lead the DMA queue); chunk 0 ships as soon as
                # kproj + vproj(0) are done, then Q immediately (so attention
                # can start), then the remaining V chunks.
                kproj(ppj, 0)
                vproj(ppj, 0)
                ship(0)
                kproj(ppj, 1)
                qproj(ppj)
                vproj(ppj, 1)
                ship(1)
                for kt in (2, 3):
                    vproj(ppj, kt)
                ship(2)
                for kt in (4, 5, 6, 7):
                    vproj(ppj, kt)
                ship(3)
            early.release()

            # ---- attention: 64 (block, kt) units, paired for DoubleRow.
            # Group 0 = the core's OWN 8 kt units straight from SBUF (no
            # collective dependency - it runs inside the AllGather latency
            # window).  The remaining 56 REMOTE units come from the gathered
            # buffers, skipping the own block via a partition-id-dependent
            # dynamic DMA offset kb = (pid+1+j) & 7.  Remote group sizes are
            # chosen so no group needs a chunk before its AllGather lands
            # (chunk availability: 7 units @AG0, +14 @AG1, +35 @AG2). ----
            pid = nc.sync.partition_id()
            kbs = [nc.sync.compute_val((pid + 1 + j) & 7) for j in range(7)]
            rem_units = [(c, j, u)
                         for c, kgl in enumerate(CHUNK_KTS)
                         for j in range(7)
                         for u in range(len(kgl))]
            # g0 = all 8 local units (fills the AllGather barrier window);
            # remote group sizes follow chunk availability
            # (7 @AG0, +7 @AG1, +14 @AG2, +28 @AG3).
            groups = [[("L", u) for u in range(KTB)]]
            for sz in (6, 8, 8, 16, 18):
                groups.append([("R",) + rem_units.pop(0) for _ in range(sz)])
            assert not rem_units

            pacc = tc.alloc_tile_pool(name="pacc", bufs=1)
            attacc = pacc.tile([P, DT, NS], F32, tag="attacc")
            acts = tc.alloc_tile_pool(name="acts", bufs=3)
            att0 = tc.alloc_tile_pool(name="att0", bufs=1)
            with (
                tc.tile_pool(name="kv", bufs=16) as kv,
                tc.tile_pool(name="ex", bufs=20) as exp_pool,
                tc.tile_pool(name="psc", bufs=2, space="PSUM") as psc,
                tc.tile_pool(name="pat", bufs=5, space="PSUM") as pat,
                tc.tile_pool(name="prs", bufs=1, space="PSUM") as prs,
            ):
                recips = []
                attn_h = []
                for gi, group in enumerate(groups):
                    first_g = gi == 0
                    last_g = gi == len(groups) - 1
                    npair = len(group) // 2
                    # assemble the group's (ktb, ktb, v_pair) sets.  A remote
                    # pair is one [P, 2, 16, 128] tile: per unit s, subtiles
                    # 0..7 hold K^T [t, k] and 8..15 hold V [d] - fetched as
                    # ONE dma with 2KB-contiguous partition lines.
                    pairs = []
                    for pi in range(npair):
                        if group[2 * pi][0] == "L":
                            u0 = group[2 * pi][1]
                            ktbs = [kts[:, :, (u0 + s) * P:(u0 + s + 1) * P]
                                    for s in range(2)]
                            vsl = (lambda dt, u0=u0:
                                   vs[:, u0:u0 + 2, dt * P:(dt + 1) * P])
                        else:
                            kvp = kv.tile([P, 2, 16, P], F8, tag="kvp",
                                          bufs=12)
                            for s in range(2):
                                _, c, j, u = group[2 * pi + s]
                                off = kbs[j] * csz[c] + 2 * USZ * u
                                nc.sync.dma_start(
                                    kvp[:, s, :, :],
                                    kvag[c][ds(off, 2 * USZ)].rearrange(
                                        "(p x) -> p x", x=2048))
                            ktbs = [kvp[:, s, 0:DT, :] for s in range(2)]
                            vsl = (lambda dt, kvp=kvp:
                                   kvp[:, :, DT + dt, :])
                        pairs.append((ktbs, vsl))
                    all_exs = []
                    for qp in range(2):
                        qpsl = slice(qp * 512, (qp + 1) * 512)
                        rs_ps = prs.tile([1, 512], F32, tag="prs")
                        exs = []
                        for pi, (ktbs, vsl) in enumerate(pairs):
                            ex = exp_pool.tile([P, 2, 512], F8, tag="ex")
                            for s in range(2):
                                sc = psc.tile([P, 512], F32, tag="psc")
                                for e2 in range(DT2):
                                    nc.tensor.matmul(
                                        sc[:],
                                        ktbs[s][:, 2 * e2:2 * e2 + 2, :],
                                        qt[:, 2 * e2:2 * e2 + 2, qpsl],
                                        start=(e2 == 0), stop=(e2 == DT2 - 1),
                                        perf_mode=DR)
                                nc.scalar.activation(ex[:, s, :], sc[:],
                                                     AF.Exp,
                                                     scale=float(SCALE))
                            exs.append(ex)
                        # denominator matmuls after all scores chains so they
                        # never wait on the ScalarE exp of their operand
                        for pi in range(npair):
                            nc.tensor.matmul(rs_ps[:], ones_p8[:, :, 0:1],
                                             exs[pi][:],
                                             start=(pi == 0),
                                             stop=(pi == npair - 1),
                                             perf_mode=DR,
                                             skip_group_check=True)
                        if first_g:
                            nc.vector.tensor_copy(rs[0:1, qpsl], rs_ps[:])
                        elif last_g:
                            # the MLP waits on rs -> recips: jump the DVE
                            # queue ahead of this group's attacc flushes.
                            with tc.high_priority():
                                nc.vector.tensor_tensor(
                                    rs[0:1, qpsl], rs_ps[:], rs[0:1, qpsl],
                                    ALU.add)
                        else:
                            nc.vector.tensor_tensor(
                                rs[0:1, qpsl], rs_ps[:], rs[0:1, qpsl],
                                ALU.add)
                        all_exs.append(exs)
                    if last_g:
                        # rs is now complete: broadcast 1/rs while the PE is
                        # busy with this group's A@V chains below.
                        # high_priority so the Tile scheduler slots the
                        # broadcast + reciprocal ahead of the A@V/attacc
                        # stream instead of after it (they gate the MLP).
                        with tc.high_priority():
                            nc.vector.tensor_copy(rs_h[:], rs[:])
                            rbs = []
                            for h in range(2):
                                rb = pat.tile([P, 512], F32, tag="pat")
                                nc.tensor.matmul(
                                    rb[:], ones_row[:],
                                    rs_h[0:1, h * 512:(h + 1) * 512])
                                rbs.append(rb)
                            for h in range(2):
                                recip = acts.tile([P, 512], F32, tag="recip",
                                                  name=f"recip{h}")
                                scr = acts.tile([P, 512], F32,
                                                tag="rscratch",
                                                name=f"rscratch{h}")
                                nc.vector.reciprocal_approx_accurate(
                                    recip[:], rbs[h][:], scr[:])
                                recips.append(recip)
                    for qp in range(2):
                        qpsl = slice(qp * 512, (qp + 1) * 512)
                        exs = all_exs[qp]
                        for dh in range(2):
                            att_ps = [pat.tile([P, 512], F32, tag="pat",
                                               name=f"att_ps{_j}")
                                      for _j in range(4)]
                            for j in range(4):
                                dt = dh * 4 + j
                                for pi, (ktbs, vsl) in enumerate(pairs):
                                    nc.tensor.matmul(
                                        att_ps[j][:],
                                        vsl(dt),
                                        exs[pi][:],
                                        start=(pi == 0),
                                        stop=(pi == npair - 1),
                                        perf_mode=DR,
                                        skip_group_check=True)
                            for j in range(4):
                                dsl = (slice(None), dh * 4 + j, qpsl)
                                if first_g:
                                    nc.vector.tensor_copy(attacc[dsl],
                                                          att_ps[j][:])
                                else:
                                    nc.vector.tensor_tensor(
                                        attacc[dsl], att_ps[j][:],
                                        attacc[dsl], ALU.add)
                        if last_g:
                            # column half qp is now complete: normalize it
                            # (straight to fp8 for the DoubleRow L1 matmul)
                            # on the DVE while the PE runs the remaining A@V
                            # chains, so the MLP starts with zero stall.
                            ah = att0.tile([P, DT, 512], F8, tag=f"y{qp}",
                                           name=f"attn_h{qp}")
                            for dt in range(DT):
                                nc.vector.tensor_tensor(
                                    ah[:, dt, :], attacc[:, dt, qpsl],
                                    recips[qp][:], ALU.mult)
                            attn_h.append(ah)

                # DMA the MLP weights (the queue drains these long before
                # the MLP starts).
                for w in ("w1T", "w2T", "w3T"):
                    nc.sync.dma_start(wT[w][:], W[w][:])

            # ---- MLP + final: layer-major, the two column halves
            # interleaved chain-by-chain so each layer boundary of one half
            # hides under matmuls of the other.  Layer 1 is fp8 DoubleRow
            # (its input is the freshly normalized fp8 attention output). ----
            with (
                tc.tile_pool(name="pml", bufs=4, space="PSUM") as pml,
                tc.tile_pool(name="outp", bufs=1) as outp,
            ):
                out_sb = outp.tile([1, NS], F32, tag="out_sb")
                cur = attn_h
                nxt = [outp.tile([P, DT, 512], F16, tag="y", bufs=4,
                                 name=f"w1y{h}") for h in range(2)]
                for h in range(2):
                    for ft in range(DT):
                        ps = pml.tile([P, 512], F32, tag="pml")
                        for e2 in range(DT2):
                            nc.tensor.matmul(
                                ps[:],
                                wT["w1T"][:, 2 * e2:2 * e2 + 2,
                                          ft * P:(ft + 1) * P],
                                cur[h][:, 2 * e2:2 * e2 + 2, :],
                                start=(e2 == 0), stop=(e2 == DT2 - 1),
                                perf_mode=DR)
                        nc.scalar.activation(
                            nxt[h][:, ft, :], ps[:],
                            AF.Relu, bias=bsb["b1"][:, ft:ft + 1])
                cur = nxt
                for wname, bname in (("w2T", "b2"), ("w3T", "b3")):
                    nxt = [outp.tile([P, DT, 512], F16, tag="y", bufs=4,
                                     name=f"{wname}y{h}") for h in range(2)]
                    for ft in range(DT):
                        for h in range(2):
                            ps = pml.tile([P, 512], F32, tag="pml")
                            for dt in range(DT):
                                nc.tensor.matmul(
                                    ps[:],
                                    wT[wname][:, dt, ft * P:(ft + 1) * P],
                                    cur[h][:, dt, :],
                                    start=(dt == 0), stop=(dt == DT - 1))
                            nc.scalar.activation(
                                nxt[h][:, ft, :], ps[:],
                                AF.Relu, bias=bsb[bname][:, ft:ft + 1])
                    cur = nxt
                for h in range(2):
                    ps = pml.tile([1, 512], F32, tag="pfin")
                    for ft in range(DT):
                        nc.tensor.matmul(
                            ps[:], fwh[:, ft:ft + 1], cur[h][:, ft, :],
                            start=(ft == 0), stop=(ft == DT - 1))
                    nc.vector.tensor_copy(
                        out_sb[0:1, h * 512:(h + 1) * 512], ps[:])
                nc.sync.dma_start(out[:], out_sb[:])
            att0.release()
            acts.release()
            pacc.release()
            kvloc.release()

    nc.compile()
    return nc


def _get_nc():
    if "nc" not in _CACHE:
        _CACHE["nc"] = _build()
    return _CACHE["nc"]


def _sbl(mT):
    """[D, cols] contraction-major -> SBUF layout [P, DT, cols]."""
    return np.ascontiguousarray(
        mT.reshape(DT, P, mT.shape[1]).transpose(1, 0, 2))


def make_in_maps(inputs):
    """Host-side sharding/layout: transpose the weights and the x shards
    into the device SBUF layout [partition, feature-tile, cols]; fp8 for
    x/Wq/Wk/Wv/W1, fp16 for MLP layers 2/3; fold bv into b1."""
    import ml_dtypes
    f32 = np.float32
    fp8 = ml_dtypes.float8_e4m3fn
    x = np.asarray(inputs["x"], dtype=f32)
    shared = {}
    for dev, ref in (("wqT", "Wq"), ("wkT", "Wk"), ("wvT", "Wv"),
                     ("w1T", "W1")):
        shared[dev] = _sbl(np.asarray(inputs[ref], dtype=f32).T.astype(fp8))
    for dev, ref in (("w2T", "W2"), ("w3T", "W3")):
        shared[dev] = _sbl(
            np.asarray(inputs[ref], dtype=f32).T.astype(np.float16))
    b1p = (np.asarray(inputs["b1"], dtype=f32)
           + np.asarray(inputs["W1"], dtype=f32)
           @ np.asarray(inputs["bv"], dtype=f32)).astype(f32)
    for dev, v in (("bq", np.asarray(inputs["bq"], dtype=f32)),
                   ("b1", b1p),
                   ("b2", np.asarray(inputs["b2"], dtype=f32)),
                   ("b3", np.asarray(inputs["b3"], dtype=f32))):
        shared[dev] = np.ascontiguousarray(v.reshape(DT, P).T)
    shared["fw"] = np.ascontiguousarray(
        np.asarray(inputs["final_weight"], dtype=f32).reshape(DT, P).T
        .astype(np.float16))
    in_maps = []
    for c in range(NCORES):
        m = dict(shared)
        xsT = _sbl(x[c * NS:(c + 1) * NS, :].T.astype(fp8))
        m["xsT0"] = np.ascontiguousarray(xsT[:, :, 0:512])
        m["xsT1"] = np.ascontiguousarray(xsT[:, :, 512:1024])
        in_maps.append(m)
    return in_maps


def kernel(**inputs):
    nc = _get_nc()
    res = bass_utils.run_bass_kernel_spmd(
        nc, make_in_maps(inputs), core_ids=list(range(NCORES)))
    return np.concatenate(
        [res.results[c]["out"].reshape(NS) for c in range(NCORES)])


# revision 31
# speedup vs baseline: 1.1357x; 1.1357x over previous
"""Trainium2 Bass kernel for DeepSelfAttention (N=8192, D=1024) on 8 NeuronCores.

Strategy (row-parallel attention, fp8 DoubleRow matmuls):
  - Shard the N=8192 rows of x across 8 cores (1024 rows each); replicate
    weights.  x and all weights are pre-arranged on the HOST into the exact
    SBUF layout [partition, feature-tile, cols] (fp8 for x/Wq/Wk/Wv/W1,
    fp16 for W2/W3), so every preamble DMA moves fully contiguous 4-8KB
    partition lines.
  - Bias algebra (host-folded):
      * bk drops out of softmax entirely (it shifts every score of a row by a
        q-dependent constant).
      * bv is folded into the first MLP bias: b1' = b1 + W1 @ bv (softmax rows
        sum to 1).
    Only bq survives on device (added to Q after the projection).
  - All projection / attention matmuls and MLP layer 1 run in fp8 with
    perf_mode=DoubleRow: operands are 3D APs [128, 2, free] contracting 256
    rows per pass, ~1.8x the fp16 matmul rate.  Host-side numpy simulation of
    this exact quantization scheme (fp8 x/Wq/Wk/Wv/W1/Q/K/V/exp, fp16 MLP
    L2/L3) gives max rel err 6.7e-3 vs the fp64 reference (gate is 2e-2);
    measured on hardware: 7.3e-3.  Full-fp8 MLP would give 2.8e-2 -> L2/L3
    stay fp16.
  - Each core computes K^T and V for its row shard and ships them in four
    fp8 chunks of [1, 1, 2, 4] key-tiles ([p][K-line|V-line]-interleaved so
    the consumer fetches a unit as ONE dma with 2KB-contiguous partition
    lines); each chunk is AllGathered as soon as it is ready.
  - The first collective carries a fixed ~50-85us rendezvous latency (core
    dispatch skew), so attention group 0 processes the core's OWN 8 key
    units straight from SBUF (kts/vs stay resident) inside that window.
    The 56 remote units are then fetched from the gathered buffers,
    skipping the own block via a partition-id-dependent dynamic DMA offset
    kb = (pid+1+j) & 7; remote group sizes (6,8,8,16,18) are matched to
    chunk arrival times so no group waits on a collective, and the late
    16/18-unit groups run 8/9-pair PSUM chains, halving the DVE
    accumulator-flush rate.
  - Attention groups pair units for DoubleRow: scores^T tiles [k=128,
    q=512] accumulate over feature-tile PAIRS in PSUM (4 DR matmuls), exp
    on ScalarE (scale=1/32 fused; no max-subtraction needed, scores are
    small) into paired fp8 tiles [128, 2, 512], softmax denominator via a
    ones-vector DR matmul, A@V accumulated across the group's unit-pairs
    (DR chains) and flushed to an SBUF fp32 accumulator.
  - Normalize via PE broadcast of 1/rowsum (emitted inside the last
    attention group, straight to fp8), then the MLP: fp8-DR layer 1, fp16
    layers 2/3 + final projection, the two 512-query column halves
    interleaved chain-by-chain to hide layer boundaries.
"""

import numpy as np

import concourse.mybir as mybir
import concourse.tile as tile
from concourse import bacc
from concourse.bass import ds
from concourse import bass_utils

P = 128
D = 1024
N = 8192
NCORES = 8
NS = N // NCORES          # 1024 rows per core
DT = D // P               # 8 feature tiles
DT2 = DT // 2             # 4 feature-tile PAIRS (DoubleRow)
KTB = NS // P             # 8 k tiles per block
CHUNK_KTS = [[0], [1], [2, 3], [4, 5, 6, 7]]  # kt split per AllGather chunk
USZ = P * D               # elements per K or V unit (128 keys x 1024 feat)
F16 = mybir.dt.float16
F32 = mybir.dt.float32
F8 = mybir.dt.float8e4
AF = mybir.ActivationFunctionType
ALU = mybir.AluOpType
DR = mybir.MatmulPerfMode.DoubleRow

SCALE = 1.0 / np.sqrt(np.float32(D)).astype(np.float32)  # 0.03125

_CACHE = {}


def _build():
    nc = bacc.Bacc("TRN2", target_bir_lowering=False, debug=False,
                   num_devices=NCORES)
    # All inputs arrive pre-arranged by the host in the exact SBUF layout
    # ([partition, feature-tile, cols]) so every preamble DMA moves fully
    # contiguous 4-8KB partition lines (~2.3x the strided-DMA rate).
    xsT = nc.dram_tensor("xsT", [P, DT, NS], F8, kind="ExternalInput").ap()
    W = {}
    for w in ("wqT", "wkT", "wvT", "w1T"):
        W[w] = nc.dram_tensor(w, [P, DT, D], F8, kind="ExternalInput").ap()
    for w in ("w2T", "w3T"):
        W[w] = nc.dram_tensor(w, [P, DT, D], F16, kind="ExternalInput").ap()
    B = {}
    for b in ("bq", "b1", "b2", "b3"):
        B[b] = nc.dram_tensor(b, [P, DT], F32, kind="ExternalInput").ap()
    fw = nc.dram_tensor("fw", [P, DT], F16, kind="ExternalInput").ap()
    out = nc.dram_tensor("out", [1, NS], F32, kind="ExternalOutput").ap()

    with tile.TileContext(nc) as tc:
        with (
            tc.tile_pool(name="persist", bufs=1) as pers,
            tc.tile_pool(name="dram", bufs=1, space="DRAM") as dram,
        ):
            # ---- persistent SBUF tiles ----
            qt = pers.tile([P, DT, NS], F8, tag="qt")           # Q^T (fp8)
            wT = {"w1T": pers.tile([P, DT, D], F8, tag="w1T", name="w1T")}
            for w in ("w2T", "w3T"):
                wT[w] = pers.tile([P, DT, D], F16, tag=f"{w}", name=f"{w}")
            bsb = {b: pers.tile([P, DT], F32, tag=f"{b}sb", name=f"{b}sb")
                   for b in B}
            fwh = pers.tile([P, DT], F16, tag="fwh")
            ones_p8 = pers.tile([P, 2, 16], F8, tag="ones8")    # DR rowsum
            ones_row = pers.tile([1, P], F16, tag="ones_row")
            rs = pers.tile([1, NS], F32, tag="rs")              # softmax denom
            rs_h = pers.tile([1, NS], F16, tag="rs_h")

            # ---- DRAM scratch: per-chunk flat [kt][K-unit | V-unit] buffers
            csz = [2 * USZ * len(k) for k in CHUNK_KTS]
            kv_d = [dram.tile([csz[c]], F8, name=f"kv_d{c}")
                    for c in range(len(CHUNK_KTS))]
            kvag = [dram.tile([NCORES * csz[c]], F8, name=f"kvag{c}",
                              addr_space="Shared")
                    for c in range(len(CHUNK_KTS))]

            # own-shard K^T / V live through attention: group 0 computes
            # attention on the local block straight from SBUF while the
            # first AllGather is still in flight.  (Allocated before the
            # early pool: pool releases must be LIFO.)
            kvloc = tc.alloc_tile_pool(name="kvloc", bufs=1)
            kts = kvloc.tile([P, DT, NS], F8, tag="kts")        # K^T shard
            vs = kvloc.tile([P, KTB, D], F8, tag="vs")          # V shard
            # ---- early pool: dies after projections ----
            early = tc.alloc_tile_pool(name="early", bufs=1)
            xsb = early.tile([P, DT, NS], F8, tag="xsb")
            wesb = {w: early.tile([P, DT, D], F8, tag=f"{w}", name=f"{w}")
                    for w in ("wqT", "wkT", "wvT")}

            # x (first half first) and the K/V weights lead the DMA queue so
            # the first projection matmul can start as early as possible.
            nc.sync.dma_start(xsb[:], xsT[:])
            nc.sync.dma_start(wesb["wkT"][:], W["wkT"][:])
            nc.sync.dma_start(wesb["wvT"][:], W["wvT"][:])
            nc.sync.dma_start(wesb["wqT"][:], W["wqT"][:])
            for b in B:
                nc.sync.dma_start(bsb[b][:], B[b][:])
            nc.sync.dma_start(fwh[:], fw[:])
            nc.gpsimd.memset(ones_p8[:], 1.0)
            nc.gpsimd.memset(ones_row[:], 1.0)

            def kproj(ppj, h):
                for dt in range(DT):
                    ps = ppj.tile([P, 512], F32, tag="ppj")
                    for e2 in range(DT2):
                        nc.tensor.matmul(
                            ps[:],
                            wesb["wkT"][:, 2 * e2:2 * e2 + 2,
                                        dt * P:(dt + 1) * P],
                            xsb[:, 2 * e2:2 * e2 + 2, h * 512:(h + 1) * 512],
                            start=(e2 == 0), stop=(e2 == DT2 - 1),
                            perf_mode=DR)
                    nc.scalar.activation(
                        kts[:, dt, h * 512:(h + 1) * 512], ps[:], AF.Copy)

            def vproj(ppj, kt):
                for dh in range(2):
                    ps = ppj.tile([P, 512], F32, tag="ppj")
                    for e2 in range(DT2):
                        nc.tensor.matmul(
                            ps[:],
                            xsb[:, 2 * e2:2 * e2 + 2, kt * P:(kt + 1) * P],
                            wesb["wvT"][:, 2 * e2:2 * e2 + 2,
                                        dh * 512:(dh + 1) * 512],
                            start=(e2 == 0), stop=(e2 == DT2 - 1),
                            perf_mode=DR)
                    nc.scalar.activation(
                        vs[:, kt, dh * 512:(dh + 1) * 512], ps[:], AF.Copy)

            def qproj(ppj):
                for dt in range(DT):
                    for h in range(2):
                        ps = ppj.tile([P, 512], F32, tag="ppj")
                        for e2 in range(DT2):
                            nc.tensor.matmul(
                                ps[:],
                                wesb["wqT"][:, 2 * e2:2 * e2 + 2,
                                            dt * P:(dt + 1) * P],
                                xsb[:, 2 * e2:2 * e2 + 2,
                                    h * 512:(h + 1) * 512],
                                start=(e2 == 0), stop=(e2 == DT2 - 1),
                                perf_mode=DR)
                        nc.vector.tensor_tensor(
                            qt[:, dt, h * 512:(h + 1) * 512], ps[:],
                            bsb["bq"][:, dt:dt + 1].to_broadcast([P, 512]),
                            ALU.add)

            def ship(c):
                # Interleaved K|V per partition: each shipped unit is laid
                # out [p][K-line 1024B | V-line 1024B] so the consumer side
                # fetches one unit as a single DMA with 2KB-contiguous
                # partition lines (twice the effective DMA rate of 1KB).
                for u, g in enumerate(CHUNK_KTS[c]):
                    uview = kv_d[c][2 * USZ * u:2 * USZ * (u + 1)].rearrange(
                        "(p x) -> p x", x=2048)
                    nc.sync.dma_start(uview[:, 0:1024],
                                      kts[:, :, g * P:(g + 1) * P])
                    nc.sync.dma_start(uview[:, 1024:2048], vs[:, g, :])
                nc.gpsimd.collective_compute(
                    "AllGather", ALU.bypass,
                    replica_groups=[list(range(NCORES))],
                    ins=[kv_d[c].opt()], outs=[kvag[c].opt()])

            with tc.tile_pool(name="ppj", bufs=4, space="PSUM") as ppj:
                # K^T = Wk @ xs^T (no bias: bk cancels in softmax);
                # V = xs @ Wv.T (bias folded into b1'). K first (its weight
                # slices lead the DMA queue); chunk 0 ships as soon as
                # kproj + vproj(0) are done, then Q immediately (so attention
                # can start), then the remaining V chunks.
                kproj(ppj, 0)
                vproj(ppj, 0)
                ship(0)
                kproj(ppj, 1)
                qproj(ppj)
                vproj(ppj, 1)
                ship(1)
                for kt in (2, 3):
                    vproj(ppj, kt)
                ship(2)
                for kt in (4, 5, 6, 7):
                    vproj(ppj, kt)
                ship(3)
            early.release()

            # ---- attention: 64 (block, kt) units, paired for DoubleRow.
            # Group 0 = the core's OWN 8 kt units straight from SBUF (no
            # collective dependency - it runs inside the AllGather latency
            # window).  The remaining 56 REMOTE units come from the gathered
            # buffers, skipping the own block via a partition-id-dependent
            # dynamic DMA offset kb = (pid+1+j) & 7.  Remote group sizes are
            # chosen so no group needs a chunk before its AllGather lands
            # (chunk availability: 7 units @AG0, +14 @AG1, +35 @AG2). ----
            pid = nc.sync.partition_id()
            kbs = [nc.sync.compute_val((pid + 1 + j) & 7) for j in range(7)]
            rem_units = [(c, j, u)
                         for c, kgl in enumerate(CHUNK_KTS)
                         for j in range(7)
                         for u in range(len(kgl))]
            # g0 = all 8 local units (fills the AllGather barrier window);
            # remote group sizes follow chunk availability
            # (7 @AG0, +7 @AG1, +14 @AG2, +28 @AG3).
            groups = [[("L", u) for u in range(KTB)]]
            for sz in (6, 8, 8, 16, 18):
                groups.append([("R",) + rem_units.pop(0) for _ in range(sz)])
            assert not rem_units

            pacc = tc.alloc_tile_pool(name="pacc", bufs=1)
            attacc = pacc.tile([P, DT, NS], F32, tag="attacc")
            acts = tc.alloc_tile_pool(name="acts", bufs=3)
            att0 = tc.alloc_tile_pool(name="att0", bufs=1)
            with (
                tc.tile_pool(name="kv", bufs=16) as kv,
                tc.tile_pool(name="ex", bufs=22) as exp_pool,
                tc.tile_pool(name="psc", bufs=2, space="PSUM") as psc,
                tc.tile_pool(name="pat", bufs=5, space="PSUM") as pat,
                tc.tile_pool(name="prs", bufs=1, space="PSUM") as prs,
            ):
                recips = []
                attn_h = []
                for gi, group in enumerate(groups):
                    first_g = gi == 0
                    last_g = gi == len(groups) - 1
                    npair = len(group) // 2
                    # assemble the group's (ktb, ktb, v_pair) sets.  A remote
                    # pair is one [P, 2, 16, 128] tile: per unit s, subtiles
                    # 0..7 hold K^T [t, k] and 8..15 hold V [d] - fetched as
                    # ONE dma with 2KB-contiguous partition lines.
                    pairs = []
                    for pi in range(npair):
                        if group[2 * pi][0] == "L":
                            u0 = group[2 * pi][1]
                            ktbs = [kts[:, :, (u0 + s) * P:(u0 + s + 1) * P]
                                    for s in range(2)]
                            vsl = (lambda dt, u0=u0:
                                   vs[:, u0:u0 + 2, dt * P:(dt + 1) * P])
                        else:
                            kvp = kv.tile([P, 2, 16, P], F8, tag="kvp",
                                          bufs=12)
                            for s in range(2):
                                _, c, j, u = group[2 * pi + s]
                                off = kbs[j] * csz[c] + 2 * USZ * u
                                nc.sync.dma_start(
                                    kvp[:, s, :, :],
                                    kvag[c][ds(off, 2 * USZ)].rearrange(
                                        "(p x) -> p x", x=2048))
                            ktbs = [kvp[:, s, 0:DT, :] for s in range(2)]
                            vsl = (lambda dt, kvp=kvp:
                                   kvp[:, :, DT + dt, :])
                        pairs.append((ktbs, vsl))
                    all_exs = []
                    for qp in range(2):
                        qpsl = slice(qp * 512, (qp + 1) * 512)
                        rs_ps = prs.tile([1, 512], F32, tag="prs")
                        exs = []
                        for pi, (ktbs, vsl) in enumerate(pairs):
                            ex = exp_pool.tile([P, 2, 512], F8, tag="ex")
                            for s in range(2):
                                sc = psc.tile([P, 512], F32, tag="psc")
                                for e2 in range(DT2):
                                    nc.tensor.matmul(
                                        sc[:],
                                        ktbs[s][:, 2 * e2:2 * e2 + 2, :],
                                        qt[:, 2 * e2:2 * e2 + 2, qpsl],
                                        start=(e2 == 0), stop=(e2 == DT2 - 1),
                                        perf_mode=DR)
                                nc.scalar.activation(ex[:, s, :], sc[:],
                                                     AF.Exp,
                                                     scale=float(SCALE))
                            exs.append(ex)
                        # denominator matmuls after all scores chains so they
                        # never wait on the ScalarE exp of their operand
                        for pi in range(npair):
                            nc.tensor.matmul(rs_ps[:], ones_p8[:, :, 0:1],
                                             exs[pi][:],
                                             start=(pi == 0),
                                             stop=(pi == npair - 1),
                                             perf_mode=DR,
                                             skip_group_check=True)
                        if first_g:
                            nc.vector.tensor_copy(rs[0:1, qpsl], rs_ps[:])
                        elif last_g:
                            # the MLP waits on rs -> recips: jump the DVE
                            # queue ahead of this group's attacc flushes.
                            with tc.high_priority():
                                nc.vector.tensor_tensor(
                                    rs[0:1, qpsl], rs_ps[:], rs[0:1, qpsl],
                                    ALU.add)
                        else:
                            nc.vector.tensor_tensor(
                                rs[0:1, qpsl], rs_ps[:], rs[0:1, qpsl],
                                ALU.add)
                        all_exs.append(exs)
                    if last_g:
                        # rs is now complete: broadcast 1/rs while the PE is
                        # busy with this group's A@V chains below.
                        # high_priority so the Tile scheduler slots the
                        # broadcast + reciprocal ahead of the A@V/attacc
                        # stream instead of after it (they gate the MLP).
                        with tc.high_priority():
                            nc.vector.tensor_copy(rs_h[:], rs[:])
                            rbs = []
                            for h in range(2):
                                rb = pat.tile([P, 512], F32, tag="pat")
                                nc.tensor.matmul(
                                    rb[:], ones_row[:],
                                    rs_h[0:1, h * 512:(h + 1) * 512])
                                rbs.append(rb)
                            for h in range(2):
                                recip = acts.tile([P, 512], F32, tag="recip",
                                                  name=f"recip{h}")
                                scr = acts.tile([P, 512], F32,
                                                tag="rscratch",
                                                name=f"rscratch{h}")
                                nc.vector.reciprocal_approx_accurate(
                                    recip[:], rbs[h][:], scr[:])
                                recips.append(recip)
                    for qp in range(2):
                        qpsl = slice(qp * 512, (qp + 1) * 512)
                        exs = all_exs[qp]
                        for dh in range(2):
                            att_ps = [pat.tile([P, 512], F32, tag="pat",
                                               name=f"att_ps{_j}")
                                      for _j in range(4)]
                            for j in range(4):
                                dt = dh * 4 + j
                                for pi, (ktbs, vsl) in enumerate(pairs):
                                    nc.tensor.matmul(
                                        att_ps[j][:],
                                        vsl(dt),
                                        exs[pi][:],
                                        start=(pi == 0),
                                        stop=(pi == npair - 1),
                                        perf_mode=DR,
                                        skip_group_check=True)
                            for j in range(4):
                                dsl = (slice(None), dh * 4 + j, qpsl)
                                if first_g:
                                    nc.vector.tensor_copy(attacc[dsl],
                                                          att_ps[j][:])
                                else:
                                    nc.vector.tensor_tensor(
                                        attacc[dsl], att_ps[j][:],
                                        attacc[dsl], ALU.add)
                        if last_g:
                            # column half qp is now complete: normalize it
                            # (straight to fp8 for the DoubleRow L1 matmul)
                            # on the DVE while the PE runs the remaining A@V
                            # chains, so the MLP starts with zero stall.
                            ah = att0.tile([P, DT, 512], F8, tag=f"y{qp}",
                                           name=f"attn_h{qp}")
                            for dt in range(DT):
                                nc.vector.tensor_tensor(
                                    ah[:, dt, :], attacc[:, dt, qpsl],
                                    recips[qp][:], ALU.mult)
                            attn_h.append(ah)

                # DMA the MLP weights (the queue drains these long before
                # the MLP starts).
                for w in ("w1T", "w2T", "w3T"):
                    nc.sync.dma_start(wT[w][:], W[w][:])

            # ---- MLP + final: layer-major, the two column halves
            # interleaved chain-by-chain so each layer boundary of one half
            # hides under matmuls of the other.  Layer 1 is fp8 DoubleRow
            # (its input is the freshly normalized fp8 attention output). ----
            with (
                tc.tile_pool(name="pml", bufs=4, space="PSUM") as pml,
                tc.tile_pool(name="outp", bufs=1) as outp,
            ):
                out_sb = outp.tile([1, NS], F32, tag="out_sb")
                cur = attn_h
                nxt = [outp.tile([P, DT, 512], F16, tag="y", bufs=4,
                                 name=f"w1y{h}") for h in range(2)]
                for h in range(2):
                    for ft in range(DT):
                        ps = pml.tile([P, 512], F32, tag="pml")
                        for e2 in range(DT2):
                            nc.tensor.matmul(
                                ps[:],
                                wT["w1T"][:, 2 * e2:2 * e2 + 2,
                                          ft * P:(ft + 1) * P],
                                cur[h][:, 2 * e2:2 * e2 + 2, :],
                                start=(e2 == 0), stop=(e2 == DT2 - 1),
                                perf_mode=DR)
                        nc.scalar.activation(
                            nxt[h][:, ft, :], ps[:],
                            AF.Relu, bias=bsb["b1"][:, ft:ft + 1])
                cur = nxt
                for wname, bname in (("w2T", "b2"), ("w3T", "b3")):
                    nxt = [outp.tile([P, DT, 512], F16, tag="y", bufs=4,
                                     name=f"{wname}y{h}") for h in range(2)]
                    for ft in range(DT):
                        for h in range(2):
                            ps = pml.tile([P, 512], F32, tag="pml")
                            for dt in range(DT):
                                nc.tensor.matmul(
                                    ps[:],
                                    wT[wname][:, dt, ft * P:(ft + 1) * P],
                                    cur[h][:, dt, :],
                                    start=(dt == 0), stop=(dt == DT - 1))
                            nc.scalar.activation(
                                nxt[h][:, ft, :], ps[:],
                                AF.Relu, bias=bsb[bname][:, ft:ft + 1])
                    cur = nxt
                for h in range(2):
                    ps = pml.tile([1, 512], F32, tag="pfin")
                    for ft in range(DT):
                        nc.tensor.matmul(
                            ps[:], fwh[:, ft:ft + 1], cur[h][:, ft, :],
                            start=(ft == 0), stop=(ft == DT - 1))
                    nc.vector.tensor_copy(
                        out_sb[0:1, h * 512:(h + 1) * 512], ps[:])
                nc.sync.dma_start(out[:], out_sb[:])
            att0.release()
            acts.release()
            pacc.release()
            kvloc.release()

    nc.compile()
    return nc


def _get_nc():
    if "nc" not in _CACHE:
        _CACHE["nc"] = _build()
    return _CACHE["nc"]


def _sbl(mT):
    """[D, cols] contraction-major -> SBUF layout [P, DT, cols]."""
    return np.ascontiguousarray(
        mT.reshape(DT, P, mT.shape[1]).transpose(1, 0, 2))


def make_in_maps(inputs):
    """Host-side sharding/layout: transpose the weights and the x shards
    into the device SBUF layout [partition, feature-tile, cols]; fp8 for
    x/Wq/Wk/Wv/W1, fp16 for MLP layers 2/3; fold bv into b1."""
    import ml_dtypes
    f32 = np.float32
    fp8 = ml_dtypes.float8_e4m3fn
    x = np.asarray(inputs["x"], dtype=f32)
    shared = {}
    for dev, ref in (("wqT", "Wq"), ("wkT", "Wk"), ("wvT", "Wv"),
                     ("w1T", "W1")):
        shared[dev] = _sbl(np.asarray(inputs[ref], dtype=f32).T.astype(fp8))
    for dev, ref in (("w2T", "W2"), ("w3T", "W3")):
        shared[dev] = _sbl(
            np.asarray(inputs[ref], dtype=f32).T.astype(np.float16))
    b1p = (np.asarray(inputs["b1"], dtype=f32)
           + np.asarray(inputs["W1"], dtype=f32)
           @ np.asarray(inputs["bv"], dtype=f32)).astype(f32)
    for dev, v in (("bq", np.asarray(inputs["bq"], dtype=f32)),
                   ("b1", b1p),
                   ("b2", np.asarray(inputs["b2"], dtype=f32)),
                   ("b3", np.asarray(inputs["b3"], dtype=f32))):
        shared[dev] = np.ascontiguousarray(v.reshape(DT, P).T)
    shared["fw"] = np.ascontiguousarray(
        np.asarray(inputs["final_weight"], dtype=f32).reshape(DT, P).T
        .astype(np.float16))
    in_maps = []
    for c in range(NCORES):
        m = dict(shared)
        m["xsT"] = _sbl(x[c * NS:(c + 1) * NS, :].T.astype(fp8))
        in_maps.append(m)
    return in_maps


def kernel(**inputs):
    nc = _get_nc()
    res = bass_utils.run_bass_kernel_spmd(
        nc, make_in_maps(inputs), core_ids=list(range(NCORES)))
    return np.concatenate(
        [res.results[c]["out"].reshape(NS) for c in range(NCORES)])


# revision 35
# speedup vs baseline: 1.1891x; 1.0471x over previous
"""Trainium2 Bass kernel for DeepSelfAttention (N=8192, D=1024) on 8 NeuronCores.

Strategy (row-parallel attention, fp8 DoubleRow matmuls):
  - Shard the N=8192 rows of x across 8 cores (1024 rows each); replicate
    weights.  x and all weights are pre-arranged on the HOST into the exact
    SBUF layout [partition, feature-tile, cols] (fp8 for x/Wq/Wk/Wv/W1,
    fp16 for W2/W3), so every preamble DMA moves fully contiguous 4-8KB
    partition lines.
  - Bias algebra (host-folded):
      * bk drops out of softmax entirely (it shifts every score of a row by a
        q-dependent constant).
      * bv is folded into the first MLP bias: b1' = b1 + W1 @ bv (softmax rows
        sum to 1).
    Only bq survives on device (added to Q after the projection).
  - All projection / attention matmuls and MLP layer 1 run in fp8 with
    perf_mode=DoubleRow: operands are 3D APs [128, 2, free] contracting 256
    rows per pass, ~1.8x the fp16 matmul rate.  Host-side numpy simulation of
    this exact quantization scheme (fp8 x/Wq/Wk/Wv/W1/Q/K/V/exp, fp16 MLP
    L2/L3) gives max rel err 6.7e-3 vs the fp64 reference (gate is 2e-2);
    measured on hardware: 7.3e-3.  Full-fp8 MLP would give 2.8e-2 -> L2/L3
    stay fp16.
  - Each core computes K^T and V for its row shard and ships them in four
    fp8 chunks of [1, 1, 2, 4] key-tiles ([p][K-line|V-line]-interleaved so
    the consumer fetches a unit as ONE dma with 2KB-contiguous partition
    lines); each chunk is AllGathered as soon as it is ready.
  - The first collective carries a fixed ~50-85us rendezvous latency (core
    dispatch skew), so attention group 0 processes the core's OWN 8 key
    units straight from SBUF (kts/vs stay resident) inside that window.
    The 56 remote units are then fetched from the gathered buffers,
    skipping the own block via a partition-id-dependent dynamic DMA offset
    kb = (pid+1+j) & 7; remote group sizes (6,8,8,16,18) are matched to
    chunk arrival times so no group waits on a collective, and the late
    16/18-unit groups run 8/9-pair PSUM chains, halving the DVE
    accumulator-flush rate.
  - Attention groups pair units for DoubleRow: scores^T tiles [k=128,
    q=512] accumulate over feature-tile PAIRS in PSUM (4 DR matmuls), exp
    on ScalarE (scale=1/32 fused; no max-subtraction needed, scores are
    small) into paired fp8 tiles [128, 2, 512], softmax denominator via a
    ones-vector DR matmul, A@V accumulated across the group's unit-pairs
    (DR chains) and flushed to an SBUF fp32 accumulator.
  - Normalize via PE broadcast of 1/rowsum (emitted inside the last
    attention group, straight to fp8), then the MLP: fp8-DR layer 1, fp16
    layers 2/3 + final projection, the two 512-query column halves
    interleaved chain-by-chain to hide layer boundaries.
"""

import numpy as np

import concourse.mybir as mybir
import concourse.tile as tile
from concourse import bacc
from concourse.bass import ds
from concourse import bass_utils

P = 128
D = 1024
N = 8192
NCORES = 8
NS = N // NCORES          # 1024 rows per core
DT = D // P               # 8 feature tiles
DT2 = DT // 2             # 4 feature-tile PAIRS (DoubleRow)
KTB = NS // P             # 8 k tiles per block
CHUNK_KTS = [[0], [1], [2, 3], [4, 5, 6, 7]]  # kt split per AllGather chunk
USZ = P * D               # elements per K or V unit (128 keys x 1024 feat)
F16 = mybir.dt.float16
F32 = mybir.dt.float32
F8 = mybir.dt.float8e4
AF = mybir.ActivationFunctionType
ALU = mybir.AluOpType
DR = mybir.MatmulPerfMode.DoubleRow

SCALE = 1.0 / np.sqrt(np.float32(D)).astype(np.float32)  # 0.03125

_CACHE = {}


def _build():
    nc = bacc.Bacc("TRN2", target_bir_lowering=False, debug=False,
                   num_devices=NCORES)
    # All inputs arrive pre-arranged by the host in the exact SBUF layout
    # ([partition, feature-tile, cols]) so every preamble DMA moves fully
    # contiguous 4-8KB partition lines (~2.3x the strided-DMA rate).
    xsT = nc.dram_tensor("xsT", [P, DT, NS], F8, kind="ExternalInput").ap()
    W = {}
    for w in ("wqT", "wkT", "wvT", "w1T", "w2T"):
        W[w] = nc.dram_tensor(w, [P, DT, D], F8, kind="ExternalInput").ap()
    W["w3T"] = nc.dram_tensor("w3T", [P, DT, D], F16,
                              kind="ExternalInput").ap()
    B = {}
    for b in ("bq", "b1", "b2", "b3"):
        B[b] = nc.dram_tensor(b, [P, DT], F32, kind="ExternalInput").ap()
    fw = nc.dram_tensor("fw", [P, DT], F16, kind="ExternalInput").ap()
    out = nc.dram_tensor("out", [1, NS], F32, kind="ExternalOutput").ap()

    with tile.TileContext(nc) as tc:
        with (
            tc.tile_pool(name="persist", bufs=1) as pers,
            tc.tile_pool(name="dram", bufs=1, space="DRAM") as dram,
        ):
            # ---- persistent SBUF tiles ----
            qt = pers.tile([P, DT, NS], F8, tag="qt")           # Q^T (fp8)
            wT = {w: pers.tile([P, DT, D], F8, tag=w, name=w)
                  for w in ("w1T", "w2T")}
            wT["w3T"] = pers.tile([P, DT, D], F16, tag="w3T", name="w3T")
            bsb = {b: pers.tile([P, DT], F32, tag=f"{b}sb", name=f"{b}sb")
                   for b in B}
            fwh = pers.tile([P, DT], F16, tag="fwh")
            ones_p8 = pers.tile([P, 2, 16], F8, tag="ones8")    # DR rowsum
            ones_row = pers.tile([1, P], F16, tag="ones_row")
            rs = pers.tile([1, NS], F32, tag="rs")              # softmax denom
            rs_h = pers.tile([1, NS], F16, tag="rs_h")

            # ---- DRAM scratch: per-chunk flat [kt][K-unit | V-unit] buffers
            csz = [2 * USZ * len(k) for k in CHUNK_KTS]
            kv_d = [dram.tile([csz[c]], F8, name=f"kv_d{c}")
                    for c in range(len(CHUNK_KTS))]
            kvag = [dram.tile([NCORES * csz[c]], F8, name=f"kvag{c}",
                              addr_space="Shared")
                    for c in range(len(CHUNK_KTS))]

            # own-shard K^T / V live through attention: group 0 computes
            # attention on the local block straight from SBUF while the
            # first AllGather is still in flight.  (Allocated before the
            # early pool: pool releases must be LIFO.)
            kvloc = tc.alloc_tile_pool(name="kvloc", bufs=1)
            kts = kvloc.tile([P, DT, NS], F8, tag="kts")        # K^T shard
            vs = kvloc.tile([P, KTB, D], F8, tag="vs")          # V shard
            # ---- early pool: dies after projections ----
            early = tc.alloc_tile_pool(name="early", bufs=1)
            xsb = early.tile([P, DT, NS], F8, tag="xsb")
            wesb = {w: early.tile([P, DT, D], F8, tag=f"{w}", name=f"{w}")
                    for w in ("wqT", "wkT", "wvT")}

            # x (first half first) and the K/V weights lead the DMA queue so
            # the first projection matmul can start as early as possible.
            nc.sync.dma_start(xsb[:], xsT[:])
            nc.sync.dma_start(wesb["wkT"][:], W["wkT"][:])
            nc.sync.dma_start(wesb["wvT"][:], W["wvT"][:])
            nc.sync.dma_start(wesb["wqT"][:], W["wqT"][:])
            for b in B:
                nc.sync.dma_start(bsb[b][:], B[b][:])
            nc.sync.dma_start(fwh[:], fw[:])
            nc.gpsimd.memset(ones_p8[:], 1.0)
            nc.gpsimd.memset(ones_row[:], 1.0)

            def kproj(ppj, h):
                for dt in range(DT):
                    ps = ppj.tile([P, 512], F32, tag="ppj")
                    for e2 in range(DT2):
                        nc.tensor.matmul(
                            ps[:],
                            wesb["wkT"][:, 2 * e2:2 * e2 + 2,
                                        dt * P:(dt + 1) * P],
                            xsb[:, 2 * e2:2 * e2 + 2, h * 512:(h + 1) * 512],
                            start=(e2 == 0), stop=(e2 == DT2 - 1),
                            perf_mode=DR)
                    nc.scalar.activation(
                        kts[:, dt, h * 512:(h + 1) * 512], ps[:], AF.Copy)

            def vproj(ppj, kt):
                for dh in range(2):
                    ps = ppj.tile([P, 512], F32, tag="ppj")
                    for e2 in range(DT2):
                        nc.tensor.matmul(
                            ps[:],
                            xsb[:, 2 * e2:2 * e2 + 2, kt * P:(kt + 1) * P],
                            wesb["wvT"][:, 2 * e2:2 * e2 + 2,
                                        dh * 512:(dh + 1) * 512],
                            start=(e2 == 0), stop=(e2 == DT2 - 1),
                            perf_mode=DR)
                    nc.scalar.activation(
                        vs[:, kt, dh * 512:(dh + 1) * 512], ps[:], AF.Copy)

            def qproj(ppj):
                for dt in range(DT):
                    for h in range(2):
                        ps = ppj.tile([P, 512], F32, tag="ppj")
                        for e2 in range(DT2):
                            nc.tensor.matmul(
                                ps[:],
                                wesb["wqT"][:, 2 * e2:2 * e2 + 2,
                                            dt * P:(dt + 1) * P],
                                xsb[:, 2 * e2:2 * e2 + 2,
                                    h * 512:(h + 1) * 512],
                                start=(e2 == 0), stop=(e2 == DT2 - 1),
                                perf_mode=DR)
                        nc.vector.tensor_tensor(
                            qt[:, dt, h * 512:(h + 1) * 512], ps[:],
                            bsb["bq"][:, dt:dt + 1].to_broadcast([P, 512]),
                            ALU.add)

            def ship(c):
                # Interleaved K|V per partition: each shipped unit is laid
                # out [p][K-line 1024B | V-line 1024B] so the consumer side
                # fetches one unit as a single DMA with 2KB-contiguous
                # partition lines (twice the effective DMA rate of 1KB).
                for u, g in enumerate(CHUNK_KTS[c]):
                    uview = kv_d[c][2 * USZ * u:2 * USZ * (u + 1)].rearrange(
                        "(p x) -> p x", x=2048)
                    nc.sync.dma_start(uview[:, 0:1024],
                                      kts[:, :, g * P:(g + 1) * P])
                    nc.sync.dma_start(uview[:, 1024:2048], vs[:, g, :])
                nc.gpsimd.collective_compute(
                    "AllGather", ALU.bypass,
                    replica_groups=[list(range(NCORES))],
                    ins=[kv_d[c].opt()], outs=[kvag[c].opt()])

            with tc.tile_pool(name="ppj", bufs=4, space="PSUM") as ppj:
                # K^T = Wk @ xs^T (no bias: bk cancels in softmax);
                # V = xs @ Wv.T (bias folded into b1'). K first (its weight
                # slices lead the DMA queue); chunk 0 ships as soon as
                # kproj + vproj(0) are done, then Q immediately (so attention
                # can start), then the remaining V chunks.
                kproj(ppj, 0)
                vproj(ppj, 0)
                ship(0)
                kproj(ppj, 1)
                qproj(ppj)
                vproj(ppj, 1)
                ship(1)
                for kt in (2, 3):
                    vproj(ppj, kt)
                ship(2)
                for kt in (4, 5, 6, 7):
                    vproj(ppj, kt)
                ship(3)
            early.release()

            # ---- attention: 64 (block, kt) units, paired for DoubleRow.
            # Group 0 = the core's OWN 8 kt units straight from SBUF (no
            # collective dependency - it runs inside the AllGather latency
            # window).  The remaining 56 REMOTE units come from the gathered
            # buffers, skipping the own block via a partition-id-dependent
            # dynamic DMA offset kb = (pid+1+j) & 7.  Remote group sizes are
            # chosen so no group needs a chunk before its AllGather lands
            # (chunk availability: 7 units @AG0, +14 @AG1, +35 @AG2). ----
            pid = nc.sync.partition_id()
            kbs = [nc.sync.compute_val((pid + 1 + j) & 7) for j in range(7)]
            rem_units = [(c, j, u)
                         for c, kgl in enumerate(CHUNK_KTS)
                         for j in range(7)
                         for u in range(len(kgl))]
            # g0 = all 8 local units (fills the AllGather barrier window);
            # remote group sizes follow chunk availability
            # (7 @AG0, +7 @AG1, +14 @AG2, +28 @AG3).
            groups = [[("L", u) for u in range(KTB)]]
            for sz in (6, 8, 8, 16, 18):
                groups.append([("R",) + rem_units.pop(0) for _ in range(sz)])
            assert not rem_units

            pacc = tc.alloc_tile_pool(name="pacc", bufs=1)
            attacc = pacc.tile([P, DT, NS], F32, tag="attacc")
            acts = tc.alloc_tile_pool(name="acts", bufs=3)
            att0 = tc.alloc_tile_pool(name="att0", bufs=1)
            with (
                tc.tile_pool(name="kv", bufs=16) as kv,
                tc.tile_pool(name="ex", bufs=22) as exp_pool,
                tc.tile_pool(name="psc", bufs=2, space="PSUM") as psc,
                tc.tile_pool(name="pat", bufs=5, space="PSUM") as pat,
                tc.tile_pool(name="prs", bufs=1, space="PSUM") as prs,
            ):
                recips = []
                attn_h = []
                for gi, group in enumerate(groups):
                    first_g = gi == 0
                    last_g = gi == len(groups) - 1
                    npair = len(group) // 2
                    # assemble the group's (ktb, ktb, v_pair) sets.  A remote
                    # pair is one [P, 2, 16, 128] tile: per unit s, subtiles
                    # 0..7 hold K^T [t, k] and 8..15 hold V [d] - fetched as
                    # ONE dma with 2KB-contiguous partition lines.
                    pairs = []
                    for pi in range(npair):
                        if group[2 * pi][0] == "L":
                            u0 = group[2 * pi][1]
                            ktbs = [kts[:, :, (u0 + s) * P:(u0 + s + 1) * P]
                                    for s in range(2)]
                            vsl = (lambda dt, u0=u0:
                                   vs[:, u0:u0 + 2, dt * P:(dt + 1) * P])
                        else:
                            kvp = kv.tile([P, 2, 16, P], F8, tag="kvp",
                                          bufs=12)
                            for s in range(2):
                                _, c, j, u = group[2 * pi + s]
                                off = kbs[j] * csz[c] + 2 * USZ * u
                                nc.sync.dma_start(
                                    kvp[:, s, :, :],
                                    kvag[c][ds(off, 2 * USZ)].rearrange(
                                        "(p x) -> p x", x=2048))
                            ktbs = [kvp[:, s, 0:DT, :] for s in range(2)]
                            vsl = (lambda dt, kvp=kvp:
                                   kvp[:, :, DT + dt, :])
                        pairs.append((ktbs, vsl))
                    all_exs = []
                    for qp in range(2):
                        qpsl = slice(qp * 512, (qp + 1) * 512)
                        rs_ps = prs.tile([1, 512], F32, tag="prs")
                        exs = []
                        for pi, (ktbs, vsl) in enumerate(pairs):
                            ex = exp_pool.tile([P, 2, 512], F8, tag="ex")
                            for s in range(2):
                                sc = psc.tile([P, 512], F32, tag="psc")
                                for e2 in range(DT2):
                                    nc.tensor.matmul(
                                        sc[:],
                                        ktbs[s][:, 2 * e2:2 * e2 + 2, :],
                                        qt[:, 2 * e2:2 * e2 + 2, qpsl],
                                        start=(e2 == 0), stop=(e2 == DT2 - 1),
                                        perf_mode=DR)
                                nc.scalar.activation(ex[:, s, :], sc[:],
                                                     AF.Exp,
                                                     scale=float(SCALE))
                            exs.append(ex)
                        # denominator matmuls after all scores chains so they
                        # never wait on the ScalarE exp of their operand
                        for pi in range(npair):
                            nc.tensor.matmul(rs_ps[:], ones_p8[:, :, 0:1],
                                             exs[pi][:],
                                             start=(pi == 0),
                                             stop=(pi == npair - 1),
                                             perf_mode=DR,
                                             skip_group_check=True)
                        if first_g:
                            nc.vector.tensor_copy(rs[0:1, qpsl], rs_ps[:])
                        elif last_g:
                            # the MLP waits on rs -> recips: jump the DVE
                            # queue ahead of this group's attacc flushes.
                            with tc.high_priority():
                                nc.vector.tensor_tensor(
                                    rs[0:1, qpsl], rs_ps[:], rs[0:1, qpsl],
                                    ALU.add)
                        else:
                            nc.vector.tensor_tensor(
                                rs[0:1, qpsl], rs_ps[:], rs[0:1, qpsl],
                                ALU.add)
                        all_exs.append(exs)
                    if last_g:
                        # rs is now complete: broadcast 1/rs while the PE is
                        # busy with this group's A@V chains below.
                        # high_priority so the Tile scheduler slots the
                        # broadcast + reciprocal ahead of the A@V/attacc
                        # stream instead of after it (they gate the MLP).
                        with tc.high_priority():
                            nc.vector.tensor_copy(rs_h[:], rs[:])
                            rbs = []
                            for h in range(2):
                                rb = pat.tile([P, 512], F32, tag="pat")
                                nc.tensor.matmul(
                                    rb[:], ones_row[:],
                                    rs_h[0:1, h * 512:(h + 1) * 512])
                                rbs.append(rb)
                            for h in range(2):
                                recip = acts.tile([P, 512], F32, tag="recip",
                                                  name=f"recip{h}")
                                scr = acts.tile([P, 512], F32,
                                                tag="rscratch",
                                                name=f"rscratch{h}")
                                nc.vector.reciprocal_approx_accurate(
                                    recip[:], rbs[h][:], scr[:])
                                recips.append(recip)
                    for qp in range(2):
                        qpsl = slice(qp * 512, (qp + 1) * 512)
                        exs = all_exs[qp]
                        for dh in range(2):
                            att_ps = [pat.tile([P, 512], F32, tag="pat",
                                               name=f"att_ps{_j}")
                                      for _j in range(4)]
                            for j in range(4):
                                dt = dh * 4 + j
                                for pi, (ktbs, vsl) in enumerate(pairs):
                                    nc.tensor.matmul(
                                        att_ps[j][:],
                                        vsl(dt),
                                        exs[pi][:],
                                        start=(pi == 0),
                                        stop=(pi == npair - 1),
                                        perf_mode=DR,
                                        skip_group_check=True)
                            for j in range(4):
                                dsl = (slice(None), dh * 4 + j, qpsl)
                                if first_g:
                                    nc.vector.tensor_copy(attacc[dsl],
                                                          att_ps[j][:])
                                else:
                                    nc.vector.tensor_tensor(
                                        attacc[dsl], att_ps[j][:],
                                        attacc[dsl], ALU.add)
                        if last_g:
                            # column half qp is now complete: normalize it
                            # (straight to fp8 for the DoubleRow L1 matmul)
                            # on the DVE while the PE runs the remaining A@V
                            # chains, so the MLP starts with zero stall.
                            ah = att0.tile([P, DT, 512], F8, tag=f"y{qp}",
                                           name=f"attn_h{qp}")
                            for dt in range(DT):
                                nc.vector.tensor_tensor(
                                    ah[:, dt, :], attacc[:, dt, qpsl],
                                    recips[qp][:], ALU.mult)
                            attn_h.append(ah)

                # DMA the MLP weights (the queue drains these long before
                # the MLP starts).
                for w in ("w1T", "w2T", "w3T"):
                    nc.sync.dma_start(wT[w][:], W[w][:])

            # ---- MLP + final: layer-major, the two column halves
            # interleaved chain-by-chain so each layer boundary of one half
            # hides under matmuls of the other.  Layer 1 is fp8 DoubleRow
            # (its input is the freshly normalized fp8 attention output). ----
            with (
                tc.tile_pool(name="pml", bufs=4, space="PSUM") as pml,
                tc.tile_pool(name="outp", bufs=1) as outp,
            ):
                out_sb = outp.tile([1, NS], F32, tag="out_sb")
                cur = attn_h
                for li, (wname, bname) in enumerate((("w1T", "b1"),
                                                     ("w2T", "b2"))):
                    # fp8 DoubleRow layer; relu output back to fp8 (layer 1)
                    # or fp16 (layer 2, feeding the fp16 layer 3)
                    odt = F8 if li == 0 else F16
                    nxt = [outp.tile([P, DT, 512], odt, tag=f"y{li}", bufs=2,
                                     name=f"{wname}y{h}") for h in range(2)]
                    for h in range(2):
                        for ft in range(DT):
                            ps = pml.tile([P, 512], F32, tag="pml")
                            for e2 in range(DT2):
                                nc.tensor.matmul(
                                    ps[:],
                                    wT[wname][:, 2 * e2:2 * e2 + 2,
                                              ft * P:(ft + 1) * P],
                                    cur[h][:, 2 * e2:2 * e2 + 2, :],
                                    start=(e2 == 0), stop=(e2 == DT2 - 1),
                                    perf_mode=DR)
                            nc.scalar.activation(
                                nxt[h][:, ft, :], ps[:],
                                AF.Relu, bias=bsb[bname][:, ft:ft + 1])
                    cur = nxt
                nxt = [outp.tile([P, DT, 512], F16, tag="y2", bufs=2,
                                 name=f"w3y{h}") for h in range(2)]
                for ft in range(DT):
                    for h in range(2):
                        ps = pml.tile([P, 512], F32, tag="pml")
                        for dt in range(DT):
                            nc.tensor.matmul(
                                ps[:],
                                wT["w3T"][:, dt, ft * P:(ft + 1) * P],
                                cur[h][:, dt, :],
                                start=(dt == 0), stop=(dt == DT - 1))
                        nc.scalar.activation(
                            nxt[h][:, ft, :], ps[:],
                            AF.Relu, bias=bsb["b3"][:, ft:ft + 1])
                cur = nxt
                for h in range(2):
                    ps = pml.tile([1, 512], F32, tag="pfin")
                    for ft in range(DT):
                        nc.tensor.matmul(
                            ps[:], fwh[:, ft:ft + 1], cur[h][:, ft, :],
                            start=(ft == 0), stop=(ft == DT - 1))
                    nc.vector.tensor_copy(
                        out_sb[0:1, h * 512:(h + 1) * 512], ps[:])
                nc.sync.dma_start(out[:], out_sb[:])
            att0.release()
            acts.release()
            pacc.release()
            kvloc.release()

    nc.compile()
    return nc


def _get_nc():
    if "nc" not in _CACHE:
        _CACHE["nc"] = _build()
    return _CACHE["nc"]


def _sbl(mT):
    """[D, cols] contraction-major -> SBUF layout [P, DT, cols]."""
    return np.ascontiguousarray(
        mT.reshape(DT, P, mT.shape[1]).transpose(1, 0, 2))


def make_in_maps(inputs):
    """Host-side sharding/layout: transpose the weights and the x shards
    into the device SBUF layout [partition, feature-tile, cols]; fp8 for
    x/Wq/Wk/Wv/W1, fp16 for MLP layers 2/3; fold bv into b1."""
    import ml_dtypes
    f32 = np.float32
    fp8 = ml_dtypes.float8_e4m3fn
    x = np.asarray(inputs["x"], dtype=f32)
    shared = {}
    for dev, ref in (("wqT", "Wq"), ("wkT", "Wk"), ("wvT", "Wv"),
                     ("w1T", "W1"), ("w2T", "W2")):
        shared[dev] = _sbl(np.asarray(inputs[ref], dtype=f32).T.astype(fp8))
    shared["w3T"] = _sbl(
        np.asarray(inputs["W3"], dtype=f32).T.astype(np.float16))
    b1p = (np.asarray(inputs["b1"], dtype=f32)
           + np.asarray(inputs["W1"], dtype=f32)
           @ np.asarray(inputs["bv"], dtype=f32)).astype(f32)
    for dev, v in (("bq", np.asarray(inputs["bq"], dtype=f32)),
                   ("b1", b1p),
                   ("b2", np.asarray(inputs["b2"], dtype=f32)),
                   ("b3", np.asarray(inputs["b3"], dtype=f32))):
        shared[dev] = np.ascontiguousarray(v.reshape(DT, P).T)
    shared["fw"] = np.ascontiguousarray(
        np.asarray(inputs["final_weight"], dtype=f32).reshape(DT, P).T
        .astype(np.float16))
    in_maps = []
    for c in range(NCORES):
        m = dict(shared)
        m["xsT"] = _sbl(x[c * NS:(c + 1) * NS, :].T.astype(fp8))
        in_maps.append(m)
    return in_maps


def kernel(**inputs):
    nc = _get_nc()
    res = bass_utils.run_bass_kernel_spmd(
        nc, make_in_maps(inputs), core_ids=list(range(NCORES)))
    return np.concatenate(
        [res.results[c]["out"].reshape(NS) for c in range(NCORES)])
